# revision 1
# baseline (speedup 1.0000x reference)
"""Self-contained Trainium2 kernel for nn_Epipolar_Attention (B=4, C=320, 32x32,
8 heads x 40). 8 NeuronCores = 4 batches x 2 query-halves, SPMD via
run_bass_kernel_spmd. See build_nc for the per-core program."""
import sys
sys.path.insert(0, '/opt/trn_rl_repo')

import numpy as np
import ml_dtypes
import bass_rust
import concourse.bass as bass
import concourse.tile as tile
from concourse import mybir
from concourse.masks import make_identity

# ---------------- walrus single-wait workaround ----------------

MAXW = 1

def _split_drain_and_barrier(self, tick_clock, wait_clock):
    nc = self.nc
    drain_bi = nc.sync.drain()
    inst = drain_bi.ins
    wait_clock.add_sem_waits(inst, bass_rust.ScopedClock({None: tick_clock.global_clock}))
    si = inst.sync_info
    waits = list(si.on_wait) if si is not None else []
    if len(waits) > MAXW:
        inst.sync_info = bass_rust.SyncInfo(on_wait=waits[:MAXW], on_update=list(si.on_update))
        rest = waits[MAXW:]
        for i in range(0, len(rest), MAXW):
            nop_bi = nc.sync.nop(nofuse=True, hint="drain_wait_split")
            nop_bi.ins.sync_info = bass_rust.SyncInfo(on_wait=rest[i:i + MAXW], on_update=[])
    nc.all_engine_barrier()
    assert self.sems is not None
    popped = nc._tile_sem_poison_stack.pop()
    assert popped is self._sem_poison
    nc.clear_and_free_semaphores(list(self.sems.allocated().values()))
    nc.all_engine_barrier()

tile.TileContext._drain_and_barrier = _split_drain_and_barrier

from concourse import mybir as _mybir

def split_multi_waits(nc):
    """Walrus in this container allows only ONE sync wait per instruction.
    Split any instruction carrying >1 waits: insert same-engine NoOps before
    it, each carrying one of the excess waits."""
    n_split = 0
    for f in nc.m.functions:
        for blk in f.blocks:
            insts = list(blk.instructions)
            out = []
            changed = False
            for inst in insts:
                si = inst.sync_info
                if si is not None and len(si.on_wait) > 1:
                    waits = list(si.on_wait)
                    for j, wv in enumerate(waits[:-1]):
                        nop = _mybir.InstNoOp(name=f"{inst.name}-ws{j}")
                        nop.engine = inst.engine
                        nop.sync_info = bass_rust.SyncInfo(on_wait=[wv], on_update=[])
                        out.append(nop)
                        n_split += 1
                    inst.sync_info = bass_rust.SyncInfo(
                        on_wait=[waits[-1]], on_update=list(si.on_update))
                    changed = True
                out.append(inst)
            if changed:
                blk.instructions = out
    return n_split


# ---------------- host geometry ----------------



def geom_features(K_in, src_c2w, tgt_c2w):
    """Host-side per-(batch,direction) geometry -> m features (HW,6), in f64.
    Replicates reference _get_epipolar up to a(q)=oi_to_pi, oi (epipole).
    Returns m (1024, 6) f64 such that distsq[q,k] = m[q] . g[k]."""
    b = K_in.shape[0]
    h, w = H, W
    Wimg = h * 16.0 / 9.0
    K = K_in.astype(np.float64) * np.array([Wimg, float(h), 1.0])[None, :, None]
    K[:, 0, 2] = h / 2.0
    K[:, 1, 2] = h / 2.0
    ii, jj = np.meshgrid(np.arange(h), np.arange(w), indexing='ij')
    coords = np.stack([jj.ravel(), ii.ravel(), np.ones(h * w)], axis=1).astype(np.float64)
    fx = K[:, 0, 0][:, None]; fy = K[:, 1, 1][:, None]
    cx = K[:, 0, 2][:, None]; cy = K[:, 1, 2][:, None]
    cam = np.stack([(coords[None, :, 0] - cx) / fx,
                    (coords[None, :, 1] - cy) / fy,
                    np.broadcast_to(coords[None, :, 2], (b, h * w))], axis=-1)
    src_r, src_t = src_c2w[:, :3, :3].astype(np.float64), src_c2w[:, :3, 3].astype(np.float64)
    tgt_r_inv = np.linalg.inv(tgt_c2w[:, :3, :3].astype(np.float64))
    tgt_t = -tgt_c2w[:, :3, 3].astype(np.float64)
    p_world = np.einsum('bij,bnj->bni', src_r, cam) + src_t[:, None]
    p_tgt = np.einsum('bij,bnj->bni', tgt_r_inv, p_world) + tgt_t[:, None]
    pi = np.einsum('bij,bnj->bni', K, p_tgt)
    pi = pi / (pi[..., -1:] + 1e-6)
    o_tgt = np.einsum('bij,bj->bi', tgt_r_inv, src_t) + tgt_t
    oi = np.einsum('bij,bj->bi', K, o_tgt)
    oi = (oi / oi[..., -1:])[:, None, :]          # (b,1,3)
    a = pi - oi                                   # (b,HW,3) oi_to_pi
    # unit direction
    ahat = a / np.linalg.norm(a, axis=-1, keepdims=True)
    # center shift: c(k) = coords(k) - oi ; c'(k) = coords(k) - center, s = center - oi
    center = np.array([ (w-1)/2.0, (h-1)/2.0, 1.0 ])   # center of pixel grid; z=1 matches coords z
    s = center[None, None, :] - oi                 # (b,1,3)
    wv = np.cross(np.broadcast_to(ahat, s.shape + (0,) * 0), 0*s) if False else None
    # w(q) = ahat(q) x s   (s constant per batch)
    wq = np.cross(ahat, np.broadcast_to(s, ahat.shape))   # (b,HW,3)
    ax, ay, az = ahat[..., 0], ahat[..., 1], ahat[..., 2]
    wx, wy, wz = wq[..., 0], wq[..., 1], wq[..., 2]
    m = np.stack([
        ay**2 + az**2,            # * cx'^2
        ax**2 + az**2,            # * cy'^2
        -2.0 * ax * ay,           # * cx'cy'
        2.0 * (az * wy - ay * wz),  # * cx'
        2.0 * (ax * wz - az * wx),  # * cy'
        (wq ** 2).sum(-1),        # * 1
    ], axis=-1)                   # (b, HW, 6)
    return m

def g_features():
    ii, jj = np.meshgrid(np.arange(H), np.arange(W), indexing='ij')
    x = jj.ravel().astype(np.float64); y = ii.ravel().astype(np.float64)
    cxp = x - (W - 1) / 2.0
    cyp = y - (H - 1) / 2.0
    g = np.stack([cxp**2, cyp**2, cxp*cyp, cxp, cyp, np.ones(HWN)], axis=-1)  # (HW, 6)
    return g



# ---------------- device program ----------------

F32 = mybir.dt.float32
F16 = mybir.dt.float16
BF16 = mybir.dt.bfloat16
AF = mybir.ActivationFunctionType
ALU = mybir.AluOpType
AX = mybir.AxisListType

B, C, H, W = 4, 320, 32, 32
HWN = H * W          # 1024
QH = HWN // 2        # 512 queries per core
HEADS, DHEAD = 8, 40
SCALE = DHEAD ** -0.5
DH41 = DHEAD + 1     # legacy
DAUG = 65            # v augmented: cols 0:40 = v, 40:64 zero, col 64 = ones (32-aligned Z row)
C_AUG = C + 1        # 321 (x-hat augmented with ones row)
D_FF = 2 * C         # 640
KT_C = [(0, 128), (128, 256), (256, 321)]       # K tiles over 321 aug channels
OG = [(0, 120), (120, 240), (240, 320)]         # och groups aligned to heads (3,3,2)
CT = [(0, 128), (128, 256), (256, 320)]         # channel tiles of 320
FT = [(0, 128), (128, 256), (256, 384), (384, 512), (512, 640)]  # 640 ff tiles


def build_nc(debug=False, reps=1):
    nc = bass.Bass(target_bir_lowering=False, debug=False)
    P = {}
    def inp(name, shape, dt):
        P[name] = nc.declare_dram_parameter(name, list(shape), dt, isOutput=False)
        return P[name]
    inp("xq", (C, QH), F32)
    inp("src", (C, HWN), F32)
    inp("gk", (36, HWN), BF16)
    inp("gc", (36, HWN), BF16)
    inp("m1q", (36, QH), BF16)
    inp("m2k", (36, HWN), BF16)
    inp("wq", (C_AUG, C), BF16)
    inp("wk", (C_AUG, C), BF16)
    inp("wv", (C_AUG, C), BF16)
    inp("wo", (HEADS, DAUG, C), BF16)
    inp("w1", (C_AUG, D_FF), BF16)
    inp("w2", (D_FF, C), BF16)
    inp("b2p", (1, C), BF16)
    inp("wpre", (128, C), F32)
    inp("wpost", (C, 1), F32)
    inp("bpost", (C, 1), F32)
    out = nc.declare_dram_parameter("out", [C, QH], F32, isOutput=True)
    dbg = {}
    if debug:
        dbg["dbg_E"] = nc.declare_dram_parameter("dbg_E", [8, 128, QH], F32, isOutput=True)
        dbg["dbg_qT"] = nc.declare_dram_parameter("dbg_qT", [HEADS, DHEAD, QH], F32, isOutput=True)
        dbg["dbg_kT"] = nc.declare_dram_parameter("dbg_kT", [HEADS, DHEAD, HWN], F32, isOutput=True)
        dbg["dbg_v"] = nc.declare_dram_parameter("dbg_v", [8, 128, HEADS * DAUG], F32, isOutput=True)
        dbg["dbg_at"] = nc.declare_dram_parameter("dbg_at", [HEADS, DAUG, QH], F32, isOutput=True)
        dbg["dbg_y"] = nc.declare_dram_parameter("dbg_y", [4, 128, C], F32, isOutput=True)

    with tile.TileContext(nc) as tc:
        for _ in range(reps):
            _emit(nc, tc, P, out, dbg)

    n = split_multi_waits(nc)
    return nc, n


def _emit(nc, tc, P, out, dbg):
    from contextlib import ExitStack
    ctx = ExitStack()
    with ctx:
        consts = ctx.enter_context(tc.tile_pool(name="consts", bufs=1))
        sbW = ctx.enter_context(tc.tile_pool(name="weights", bufs=1))
        sbP = ctx.enter_context(tc.tile_pool(name="persist", bufs=1))
        sbT = ctx.enter_context(tc.tile_pool(name="scratch", bufs=2))
        sbIn = ctx.enter_context(tc.tile_pool(name="inputs", bufs=1))
        drB = ctx.enter_context(tc.tile_pool(name="dram", bufs=2, space="DRAM"))

        ident = consts.tile([128, 128], F32)
        make_identity(nc, ident[:])
        epst = consts.tile([128, 1], F32)
        nc.vector.memset(epst[:], 1e-5)
        b3e4 = consts.tile([128, 1], F32)
        nc.vector.memset(b3e4[:], 1.5e-3)
        c25 = consts.tile([128, 1], F32)
        nc.vector.memset(c25[:], 25.0)
        ones1 = consts.tile([1, 128], BF16)
        nc.vector.memset(ones1[:], 1.0)
        ones65 = consts.tile([1, DAUG], BF16)
        nc.vector.memset(ones65[:], 1.0)

        # ---- load small inputs ----
        def load(name, shape, dt, pool=sbW):
            t = pool.tile(list(shape), dt, name=name, tag=name)
            nc.sync.dma_start(out=t[:], in_=P[name][:])
            return t
        gk = load("gk", (36, HWN), BF16)
        gc = load("gc", (36, HWN), BF16)
        m1q = load("m1q", (36, QH), BF16)
        m2k = load("m2k", (36, HWN), BF16)
        wq_sb = [load_slice(nc, sbW, P["wq"], kt, C, BF16) for kt in KT_C]
        wk_sb = [load_slice(nc, sbW, P["wk"], kt, C, BF16) for kt in KT_C]
        wv_sb = [load_slice(nc, sbW, P["wv"], kt, C, BF16) for kt in KT_C]
        wo_sb = []
        for h in range(HEADS):
            t = sbW.tile([DAUG, C], BF16, name=f"wo{h}", tag=f"wo{h}")
            nc.sync.dma_start(out=t[:], in_=P["wo"][h, :, :])
            wo_sb.append(t)
        w1_sb = [load_slice(nc, sbW, P["w1"], kt, D_FF, BF16) for kt in KT_C]
        w2_sb = []
        for ft in FT:
            t = sbW.tile([128, C], BF16, name=f"w2{ft[0]}", tag=f"w2{ft[0]}")
            nc.sync.dma_start(out=t[:], in_=P["w2"][ft[0]:ft[1], :])
            w2_sb.append(t)
        b2p = load("b2p", (1, C), BF16)
        wpre = load("wpre", (128, C), F32)
        wpost_sb = [load_slice(nc, sbW, P["wpost"], ct, 1, F32) for ct in CT]
        bpost_sb = [load_slice(nc, sbW, P["bpost"], ct, 1, F32) for ct in CT]

        # =========== Phase A: epipolar mask ===========
        E = [sbP.tile([128, QH], BF16, name=f"E{kt}", tag=f"E{kt}") for kt in range(8)]
        dist1 = [sbP.tile([128, QH], F16, name=f"ds1_{i}", tag=f"ds1_{i}") for i in range(8)]
        dist2 = [sbP.tile([128, QH], F16, name=f"ds2_{i}", tag=f"ds2_{i}") for i in range(8)]
        flag2 = [sbP.tile([128, 1], F32, name=f"fl2_{i}", tag=f"fl2_{i}") for i in range(8)]
        flags1 = sbP.tile([128, 4], BF16)

        with tc.tile_pool(name="psA", bufs=2, space="PSUM") as psA:
            # flag1: distsq1 in [q, k] layout, min over k
            for qt in range(4):
                d1 = psA.tile([128, HWN], F32, name="dA", tag="dA")
                nc.tensor.matmul(d1[:, 0:512], m1q[:, qt*128:(qt+1)*128], gk[:, 0:512],
                                 start=True, stop=True)
                nc.tensor.matmul(d1[:, 512:1024], m1q[:, qt*128:(qt+1)*128], gk[:, 512:1024],
                                 start=True, stop=True)
                mn = sbT.tile([128, 1], F32, name="mn", tag="mn")
                nc.vector.tensor_reduce(out=mn[:], in_=d1[:], axis=AX.X, op=ALU.min)
                nc.vector.tensor_scalar(out=flags1[:, qt:qt+1], in0=mn[:], scalar1=0.25,
                                        scalar2=None, op0=ALU.is_gt)
            # distsq2 in [k, c] layout: flag2 (min over all c) + dist2 (cols 0:512)
            for kt in range(8):
                d2 = psA.tile([128, HWN], F32, name="dA", tag="dA")
                nc.tensor.matmul(d2[:, 0:512], m2k[:, kt*128:(kt+1)*128], gc[:, 0:512],
                                 start=True, stop=True)
                nc.tensor.matmul(d2[:, 512:1024], m2k[:, kt*128:(kt+1)*128], gc[:, 512:1024],
                                 start=True, stop=True)
                mn2 = sbT.tile([128, 1], F32, name="mn", tag="mn")
                nc.vector.tensor_reduce(out=mn2[:], in_=d2[:], axis=AX.X, op=ALU.min)
                nc.vector.tensor_scalar(out=flag2[kt][:], in0=mn2[:], scalar1=0.25,
                                        scalar2=None, op0=ALU.is_gt)
                nc.scalar.activation(out=dist2[kt][:], in_=d2[:, 0:512], func=AF.Sqrt,
                                     bias=b3e4[:], scale=1.0)
            # distsq1 in [k, q] layout -> dist1
            for kt in range(8):
                d1t = psA.tile([128, QH], F32, name="dB", tag="dB")
                nc.tensor.matmul(d1t[:], gk[:, kt*128:(kt+1)*128], m1q[:],
                                 start=True, stop=True)
                nc.scalar.activation(out=dist1[kt][:], in_=d1t[:], func=AF.Sqrt,
                                     bias=b3e4[:], scale=1.0)

        # flag1 row -> dram -> broadcast tile
        fl_d = drB.tile([1, QH], BF16)
        out_ap = bass.AP(tensor=fl_d.tensor, offset=fl_d.offset, ap=[[1, 128], [128, 4]])
        nc.sync.dma_start(out=out_ap, in_=flags1[:])
        flag1b = sbP.tile([128, QH], BF16)
        bc_ap = bass.AP(tensor=fl_d.tensor, offset=fl_d.offset, ap=[[0, 128], [1, QH]])
        nc.sync.dma_start(out=flag1b[:], in_=bc_ap)

        # sigmoids (one ACT table set) + flag fix + product
        for kt in range(8):
            dw1 = sbT.tile([128, QH], BF16, name="dw1", tag="dw1")
            nc.scalar.activation(out=dw1[:], in_=dist1[kt][:], func=AF.Sigmoid,
                                 bias=c25[:], scale=-50.0)
            dw2 = sbT.tile([128, QH], BF16, name="dw2", tag="dw2")
            nc.scalar.activation(out=dw2[:], in_=dist2[kt][:], func=AF.Sigmoid,
                                 bias=c25[:], scale=-50.0)
            e2f = sbT.tile([128, QH], BF16, name="e2f", tag="e2f")
            nc.vector.tensor_scalar(out=e2f[:], in0=dw2[:], scalar1=flag2[kt][:],
                                    scalar2=None, op0=ALU.max)
            e1f = sbT.tile([128, QH], BF16, name="e1f", tag="e1f")
            nc.vector.tensor_tensor(out=e1f[:], in0=dw1[:], in1=flag1b[:], op=ALU.max)
            nc.vector.tensor_tensor(out=E[kt][:], in0=e1f[:], in1=e2f[:], op=ALU.mult)
            if dbg:
                ef32 = sbT.tile([128, QH], F32, name="ef32", tag="ef32")
                nc.vector.tensor_copy(out=ef32[:], in_=E[kt][:])
                nc.gpsimd.dma_start(out=dbg["dbg_E"][kt, :, :], in_=ef32[:])

        # =========== Phase B: LN + projections ===========
        xhatT = _ln_and_transpose(nc, tc, sbP, sbT, sbIn, P["xq"], QH, ident, epst, "xh")
        srcT = _ln_and_transpose(nc, tc, sbP, sbT, sbIn, P["src"], HWN, ident, epst, "sh")

        qT = [sbP.tile([DHEAD, QH], BF16, name=f"qT{h}", tag=f"qT{h}") for h in range(HEADS)]
        kT = [sbP.tile([DHEAD, HWN], BF16, name=f"kT{h}", tag=f"kT{h}") for h in range(HEADS)]
        v_sb = [sbP.tile([128, HEADS, DAUG], BF16, name=f"v{pt}", tag=f"v{pt}") for pt in range(8)]

        with tc.tile_pool(name="psB", bufs=2, space="PSUM") as psB:
            for h in range(HEADS):
                o0, o1 = h * DHEAD, (h + 1) * DHEAD
                qp = psB.tile([DHEAD, QH], F32, name="qk", tag="qk")
                for ki, (k0, k1) in enumerate(KT_C):
                    nc.tensor.matmul(qp[:], wq_sb[ki][:, o0:o1], xhatT[ki][:],
                                     start=(ki == 0), stop=(ki == 2))
                nc.any.tensor_copy(out=qT[h][:], in_=qp[:])
                kp = psB.tile([DHEAD, HWN], F32, name="kk", tag="kk")
                for ki, (k0, k1) in enumerate(KT_C):
                    nc.tensor.matmul(kp[:, 0:512], wk_sb[ki][:, o0:o1], srcT[ki][:, 0:512],
                                     start=(ki == 0), stop=(ki == 2))
                    nc.tensor.matmul(kp[:, 512:1024], wk_sb[ki][:, o0:o1], srcT[ki][:, 512:1024],
                                     start=(ki == 0), stop=(ki == 2))
                nc.any.tensor_copy(out=kT[h][:], in_=kp[:])
            for pt in range(8):
                vp = psB.tile([128, C], F32, name="vp", tag="vp")
                for ki, (k0, k1) in enumerate(KT_C):
                    nc.tensor.matmul(vp[:], srcT[ki][:, pt*128:(pt+1)*128], wv_sb[ki][:],
                                     start=(ki == 0), stop=(ki == 2))
                nc.gpsimd.memset(v_sb[pt][:, :, 40:64], 0.0)
                nc.gpsimd.memset(v_sb[pt][:, :, 64:65], 1.0)
                nc.any.tensor_copy(out=v_sb[pt][:, :, 0:40],
                                      in_=vp[:].rearrange("p (h d) -> p h d", h=HEADS))
                if dbg:
                    vf = sbT.tile([128, HEADS * DH41], F32, name="vf", tag="vf")
                    nc.vector.tensor_copy(out=vf[:], in_=v_sb[pt][:].rearrange("p h d -> p (h d)"))
                    nc.gpsimd.dma_start(out=dbg["dbg_v"][pt, :, :], in_=vf[:])
        if dbg:
            for h in range(HEADS):
                qf = sbT.tile([DHEAD, QH], F32, name="qf", tag="qf")
                nc.vector.tensor_copy(out=qf[:], in_=qT[h][:])
                nc.gpsimd.dma_start(out=dbg["dbg_qT"][h, :, :], in_=qf[:])
                kf = sbT.tile([DHEAD, HWN], F32, name="kf", tag="kf")
                nc.vector.tensor_copy(out=kf[:], in_=kT[h][:])
                nc.gpsimd.dma_start(out=dbg["dbg_kT"][h, :, :], in_=kf[:])

        # =========== Phase C: attention ===========
        resid1 = [sbP.tile([128, C], F32, name=f"res{pt}", tag=f"res{pt}") for pt in range(4)]
        zT = [sbP.tile([r1 - r0 + (1 if i == 2 else 0), QH], BF16, name=f"zT{i}", tag=f"zT{i}")
              for i, (r0, r1) in enumerate(CT)]
        nc.vector.memset(zT[2][64:65, :], 1.0)

        atn_all = [sbP.tile([DAUG, QH], BF16, name=f"atn{h}", tag=f"atn{h}")
                   for h in range(HEADS)]
        with (
            tc.tile_pool(name="psY", bufs=2, space="PSUM") as psY,
            tc.tile_pool(name="psSt", bufs=2, space="PSUM") as psSt,
            tc.tile_pool(name="psAt", bufs=2, space="PSUM") as psAt,
            tc.tile_pool(name="psTp", bufs=1, space="PSUM") as psTp,
            tc.tile_pool(name="psZb", bufs=1, space="PSUM") as psZb,
        ):
            for h in range(HEADS):
                pin = sbT.tile([128, 8, QH], BF16, name="pin", tag="pin")
                for kt in range(8):
                    st = psSt.tile([128, QH], F32, name="st", tag="st")
                    nc.tensor.matmul(st[:], kT[h][:, kt*128:(kt+1)*128], qT[h][:],
                                     start=True, stop=True)
                    nc.vector.tensor_tensor(out=pin[:, kt, :], in0=st[:], in1=E[kt][:], op=ALU.mult)
                pk = sbT.tile([128, 8, QH], BF16, name="pew", tag="pew")
                nc.scalar.activation(out=pk[:], in_=pin[:], func=AF.Exp, bias=0.0, scale=1.0)
                at = psAt.tile([DAUG, QH], F32, name="at", tag="at")
                for kt in range(8):
                    nc.tensor.matmul(at[:], v_sb[kt][:, h, :], pk[:, kt, :],
                                     start=(kt == 0), stop=(kt == 7))
                invz = sbT.tile([1, QH], BF16, name="invz", tag="invz")
                with nc.allow_low_precision(reason="invZ row scale cancels in LN_pre"):
                    nc.vector.reciprocal(out=invz[:], in_=at[64:65, :])
                zb = psZb.tile([DAUG, QH], F32, name="zb", tag="zb")
                nc.tensor.matmul(zb[:], ones65[:], invz[:], start=True, stop=True)
                at_sb = sbT.tile([DAUG, QH], BF16, name="at_sb", tag="at_sb")
                nc.any.tensor_copy(out=at_sb[:], in_=at[:])
                atn = atn_all[h]
                nc.vector.tensor_tensor(out=atn[:], in0=at_sb[:], in1=zb[:], op=ALU.mult)
                if dbg:
                    af = sbT.tile([DAUG, QH], F32, name="af", tag="af")
                    nc.vector.tensor_copy(out=af[:], in_=atn[:])
                    nc.gpsimd.dma_start(out=dbg["dbg_at"][h, :, :], in_=af[:])
            # =========== Phase D: Wo GEMM + LN_pre + transpose z ===========
            for pt in range(4):
                y_ps = psY.tile([128, C], F32, name="yp", tag="yp")
                for h in range(HEADS):
                    nc.tensor.matmul(y_ps[:], atn_all[h][:, pt*128:(pt+1)*128], wo_sb[h][:],
                                     start=(h == 0), stop=(h == HEADS - 1))
                if dbg:
                    yf = sbT.tile([128, C], F32, name="yf", tag="yf")
                    nc.vector.tensor_copy(out=yf[:], in_=y_ps[:])
                    nc.gpsimd.dma_start(out=dbg["dbg_y"][pt, :, :], in_=yf[:])
                zhat = _ln_apply(nc, sbT, y_ps, C, epst, "z")
                nc.vector.tensor_tensor(out=resid1[pt][:], in0=zhat[:], in1=wpre[:], op=ALU.mult)
                for ci, (c0, c1) in enumerate(CT):
                    cw = c1 - c0
                    tp = psTp.tile([128, 128], F32, name="tp", tag="tp")
                    nc.tensor.transpose(tp[0:cw, 0:128], zhat[:, c0:c1], ident[:])
                    nc.any.tensor_copy(out=zT[ci][0:cw, pt*128:(pt+1)*128], in_=tp[0:cw, 0:128])

        # MLP
        g1 = [sbP.tile([128, QH], BF16, name=f"g1{mt}", tag=f"g1{mt}") for mt in range(5)]
        with (
            tc.tile_pool(name="psM", bufs=2, space="PSUM") as psM,
            tc.tile_pool(name="psV", bufs=1, space="PSUM") as psV,
        ):
            for mt, (f0, f1) in enumerate(FT):
                h1 = psM.tile([128, QH], F32, name="h1", tag="h1")
                for ki in range(3):
                    nc.tensor.matmul(h1[:], w1_sb[ki][:, f0:f1], zT[ki][:],
                                     start=(ki == 0), stop=(ki == 2))
                nc.scalar.activation(out=g1[mt][:], in_=h1[:], func=AF.Gelu, bias=0.0, scale=1.0)
            vt_ps = [psV.tile([128, QH], F32, name=f"vt{ci}", tag=f"vt{ci}") for ci in range(3)]
            for pt in range(4):
                mp = psM.tile([128, C], F32, name="mp", tag="mp")
                nc.tensor.matmul(mp[:], ones1[:, 0:128], b2p[:],
                                 start=True, stop=False)
                for mt in range(5):
                    nc.tensor.matmul(mp[:], g1[mt][:, pt*128:(pt+1)*128], w2_sb[mt][:],
                                     start=False, stop=(mt == 4))
                res = sbT.tile([128, C], F32, name="res2", tag="res2")
                nc.vector.tensor_tensor(out=res[:], in0=resid1[pt][:], in1=mp[:], op=ALU.add)
                vhat = _ln_apply(nc, sbT, res, C, epst, "v2")
                for ci, (c0, c1) in enumerate(CT):
                    cw = c1 - c0
                    nc.tensor.transpose(vt_ps[ci][0:cw, pt*128:(pt+1)*128], vhat[:, c0:c1], ident[:])
            for ci, (c0, c1) in enumerate(CT):
                cw = c1 - c0
                o_sb = sbT.tile([128, QH], F32, name="osb", tag="osb")
                nc.vector.tensor_scalar(out=o_sb[0:cw, :], in0=vt_ps[ci][0:cw, :],
                                        scalar1=wpost_sb[ci][0:cw, :], scalar2=bpost_sb[ci][0:cw, :],
                                        op0=ALU.mult, op1=ALU.add)
                nc.sync.dma_start(out=out[c0:c1, :], in_=o_sb[0:cw, :])


def load_slice(nc, pool, param, kt, ncols, dt):
    k0, k1 = kt
    t = pool.tile([k1 - k0, ncols], dt, name=f"ld{param.name}{k0}", tag=f"ld{param.name}{k0}")
    nc.sync.dma_start(out=t[:], in_=param[k0:k1, :])
    return t


def _ln_apply(nc, sbT, x_ps, nfree, epst, tag):
    """LN normalize (no affine) along free axis from a PSUM/SBUF tile [128, nfree]."""
    stats = sbT.tile([128, 6], F32, name=f"st{tag}", tag=f"st{tag}")
    nc.vector.bn_stats(out=stats[:], in_=x_ps[:])
    mv = sbT.tile([128, 2], F32, name=f"mv{tag}", tag=f"mv{tag}")
    nc.vector.bn_aggr(out=mv[:], in_=stats[:])
    sd = sbT.tile([128, 1], F32, name=f"sd{tag}", tag=f"sd{tag}")
    nc.scalar.activation(out=sd[:], in_=mv[:, 1:2], func=AF.Sqrt, bias=epst[:], scale=1.0)
    rstd = sbT.tile([128, 1], F32, name=f"rs{tag}", tag=f"rs{tag}")
    nc.vector.reciprocal(out=rstd[:], in_=sd[:])
    negms = sbT.tile([128, 1], F32, name=f"nm{tag}", tag=f"nm{tag}")
    nc.vector.tensor_scalar(out=negms[:], in0=mv[:, 0:1], scalar1=rstd[:], scalar2=-1.0,
                            op0=ALU.mult, op1=ALU.mult)
    xhat = sbT.tile([128, nfree], F32, name=f"xh{tag}", tag=f"xh{tag}")
    nc.scalar.activation(out=xhat[:], in_=x_ps[:], func=AF.Identity, bias=negms[:], scale=rstd[:])
    return xhat


def _ln_and_transpose(nc, tc, sbP, sbT, sbIn, xparam, npix, ident, epst, tag):
    """Load [C, npix] channel-major input, LN per pixel over channels, return
    augmented channel-major tiles [128,128,65] x npix with ones row 64 of tile 2."""
    ntiles = npix // 128
    x_sb = []
    for ci, (c0, c1) in enumerate(CT):
        t = sbIn.tile([c1 - c0, npix], F32, name=f"x{tag}{ci}", tag=f"x{tag}{ci}")
        nc.sync.dma_start(out=t[:], in_=xparam[c0:c1, :])
        x_sb.append(t)
    outT = [sbP.tile([(k1 - k0), npix], BF16, name=f"xT{tag}{i}", tag=f"xT{tag}{i}")
            for i, (k0, k1) in enumerate(KT_C)]
    nc.vector.memset(outT[2][64:65, :], 1.0)
    with tc.tile_pool(name=f"psT{tag}", bufs=2, space="PSUM") as psT:
        for pt in range(ntiles):
            xp = psT.tile([128, C], F32, name="xp", tag="xp")
            for ci, (c0, c1) in enumerate(CT):
                cw = c1 - c0
                nc.tensor.transpose(xp[:, c0:c1], x_sb[ci][:, pt*128:(pt+1)*128],
                                    ident[0:cw, 0:cw])
            xhat = _ln_apply(nc, sbT, xp, C, epst, tag)
            for ci, (c0, c1) in enumerate(CT):
                cw = c1 - c0
                tb = psT.tile([128, 128], F32, name="tb", tag="tb")
                nc.tensor.transpose(tb[0:cw, 0:128], xhat[:, c0:c1], ident[:])
                nc.any.tensor_copy(out=outT[ci][0:cw, pt*128:(pt+1)*128], in_=tb[0:cw, 0:128])
    return outT


# ================= host side =================

def host_prep(x, src_encode, intrinsic, c2w, ln_q_w, ln_q_b, Wq, bq, ln_k_w, ln_k_b,
              Wk, bk, ln_v_w, ln_v_b, Wv, bv, Wo, bo, ln_pre_w, ln_pre_b, W1, b1,
              W2, b2, ln_post_w, ln_post_b):
    """Returns list of 8 in_maps."""
    g = g_features()                                   # (1024, 6) f64
    m1 = geom_features(intrinsic, c2w[1], c2w[0])      # (B, 1024, 6)
    m2 = geom_features(intrinsic, c2w[0], c2w[1])

    def split3(a):
        hi = a.astype(ml_dtypes.bfloat16).astype(np.float64)
        mid = (a - hi).astype(ml_dtypes.bfloat16).astype(np.float64)
        lo = (a - hi - mid).astype(ml_dtypes.bfloat16).astype(np.float64)
        return hi, mid, lo

    def feat36(m, g_):
        mh, mm, ml = split3(m)
        gh, gm, gl = split3(g_)
        mrows = np.concatenate([mh, mm, mh, ml, mh, mm], axis=-1)
        grows = np.concatenate([gh, gh, gm, gh, gl, gm], axis=-1)
        return mrows, grows

    f64 = np.float64
    Wq_f = (np.diag(ln_q_w.astype(f64)) @ Wq.astype(f64)) * SCALE
    bq_f = (ln_q_b.astype(f64) @ Wq.astype(f64) + bq) * SCALE
    Wk_f = np.diag(ln_k_w.astype(f64)) @ Wk.astype(f64)
    bk_f = ln_k_b.astype(f64) @ Wk.astype(f64) + bk
    Wv_f = np.diag(ln_v_w.astype(f64)) @ Wv.astype(f64)
    bv_f = ln_v_b.astype(f64) @ Wv.astype(f64) + bv
    W1_f = np.diag(ln_pre_w.astype(f64)) @ W1.astype(f64)
    b1_f = ln_pre_b.astype(f64) @ W1.astype(f64) + b1
    b2p = b2.astype(f64) + ln_pre_b.astype(f64)

    def aug(Wf, bf):
        return np.concatenate([Wf, bf[None, :]], 0).astype(ml_dtypes.bfloat16)

    wq_np = aug(Wq_f, bq_f)
    wk_np = aug(Wk_f, bk_f)
    wv_np = aug(Wv_f, bv_f)
    w1_np = aug(W1_f, b1_f)
    wo_np = np.zeros((HEADS, 65, C), np.float64)
    for h in range(HEADS):
        wo_np[h, 0:DHEAD, :] = Wo[h*DHEAD:(h+1)*DHEAD, :]
    wo_np[0, 64, :] = bo
    wo_np = wo_np.astype(ml_dtypes.bfloat16)
    w2_np = W2.astype(ml_dtypes.bfloat16)
    b2p_np = b2p[None, :].astype(ml_dtypes.bfloat16)
    wpre_np = np.broadcast_to(ln_pre_w[None, :], (128, C)).astype(np.float32).copy()
    wpost_np = ln_post_w[:, None].astype(np.float32).copy()
    bpost_np = ln_post_b[:, None].astype(np.float32).copy()

    in_maps = []
    for core in range(8):
        b = core // 2
        half = core % 2
        qsel = np.arange(half * QH, (half + 1) * QH)
        other = np.arange((1 - half) * QH, (2 - half) * QH)
        perm = np.concatenate([qsel, other])
        m1rows, g1rows = feat36(m1[b][qsel], g)
        m2rows, g2rows = feat36(m2[b], g[perm])
        bf = ml_dtypes.bfloat16
        im = {
            "xq": np.ascontiguousarray(x[b].reshape(C, HWN)[:, qsel].astype(np.float32)),
            "src": np.ascontiguousarray(src_encode[b].reshape(C, HWN).astype(np.float32)),
            "gk": np.ascontiguousarray(g1rows.T.astype(bf)),
            "gc": np.ascontiguousarray(g2rows.T.astype(bf)),
            "m1q": np.ascontiguousarray(m1rows.T.astype(bf)),
            "m2k": np.ascontiguousarray(m2rows.T.astype(bf)),
            "wq": wq_np, "wk": wk_np, "wv": wv_np, "wo": wo_np,
            "w1": w1_np, "w2": w2_np, "b2p": b2p_np,
            "wpre": wpre_np, "wpost": wpost_np, "bpost": bpost_np,
        }
        in_maps.append(im)
    return in_maps


def assemble(results):
    out = np.zeros((B, C, HWN), np.float32)
    for core in range(8):
        b, half = core // 2, core % 2
        out[b][:, half*QH:(half+1)*QH] = results[core]["out"]
    return out.reshape(B, C, H, W)


_CACHE = {}

def kernel(**inputs):
    from concourse.bass_utils import run_bass_kernel_spmd
    if "nc" not in _CACHE:
        _CACHE["nc"], _ = build_nc(debug=False)
    nc = _CACHE["nc"]
    in_maps = host_prep(**inputs)
    res = run_bass_kernel_spmd(nc, in_maps, core_ids=list(range(8)))
    return assemble(res.results)



# revision 29
# speedup vs baseline: 211.3887x; 211.3887x over previous
"""Self-contained Trainium2 kernel for nn_Epipolar_Attention (B=4, C=320, 32x32,
8 heads x 40). 8 NeuronCores = 4 batches x 2 query-halves, SPMD via
run_bass_kernel_spmd.

v2: epipolar mask computed host-side (exact), shipped as f16; pair-packed
projections; row-tiled QK^T (2 heads concurrent in PE); exp from PSUM on
scalar; pk-1 = (exp(st)-1)*E blend on vector with the +1 folded into a
V1 = sum_k v column-sum init matmul; col-tiled PV; row-tiled Wo;
Quake-rsqrt LayerNorm on vector (no Sqrt act-table loads); LN applies on
gpsimd."""
import sys
sys.path.insert(0, '/opt/trn_rl_repo')

import numpy as np
import ml_dtypes
import bass_rust
import concourse.bass as bass
import concourse.tile as tile
from concourse import mybir
from concourse.masks import make_identity

# ---------------- walrus single-wait workaround ----------------

MAXW = 1

def _split_drain_and_barrier(self, tick_clock, wait_clock):
    nc = self.nc
    drain_bi = nc.sync.drain()
    inst = drain_bi.ins
    wait_clock.add_sem_waits(inst, bass_rust.ScopedClock({None: tick_clock.global_clock}))
    si = inst.sync_info
    waits = list(si.on_wait) if si is not None else []
    if len(waits) > MAXW:
        inst.sync_info = bass_rust.SyncInfo(on_wait=waits[:MAXW], on_update=list(si.on_update))
        rest = waits[MAXW:]
        for i in range(0, len(rest), MAXW):
            nop_bi = nc.sync.nop(nofuse=True, hint="drain_wait_split")
            nop_bi.ins.sync_info = bass_rust.SyncInfo(on_wait=rest[i:i + MAXW], on_update=[])
    nc.all_engine_barrier()
    assert self.sems is not None
    popped = nc._tile_sem_poison_stack.pop()
    assert popped is self._sem_poison
    nc.clear_and_free_semaphores(list(self.sems.allocated().values()))
    nc.all_engine_barrier()

tile.TileContext._drain_and_barrier = _split_drain_and_barrier

from concourse import mybir as _mybir

def split_multi_waits(nc):
    """Walrus in this container allows only ONE sync wait per instruction.
    Split any instruction carrying >1 waits: insert same-engine NoOps before
    it, each carrying one of the excess waits."""
    n_split = 0
    for f in nc.m.functions:
        for blk in f.blocks:
            insts = list(blk.instructions)
            out = []
            changed = False
            for inst in insts:
                si = inst.sync_info
                if si is not None and len(si.on_wait) > 1:
                    waits = list(si.on_wait)
                    for j, wv in enumerate(waits[:-1]):
                        nop = _mybir.InstNoOp(name=f"{inst.name}-ws{j}")
                        nop.engine = inst.engine
                        nop.sync_info = bass_rust.SyncInfo(on_wait=[wv], on_update=[])
                        out.append(nop)
                        n_split += 1
                    inst.sync_info = bass_rust.SyncInfo(
                        on_wait=[waits[-1]], on_update=list(si.on_update))
                    changed = True
                out.append(inst)
            if changed:
                blk.instructions = out
    return n_split


# ---------------- constants ----------------

F32 = mybir.dt.float32
F16 = mybir.dt.float16
I32 = mybir.dt.int32
AF = mybir.ActivationFunctionType
ALU = mybir.AluOpType
AX = mybir.AxisListType

B, C, H, W = 4, 320, 32, 32
HWN = H * W          # 1024
QH = HWN // 2        # 512 queries per core
HEADS, DHEAD = 8, 40
SCALE = DHEAD ** -0.5
C_AUG = C + 1        # 321 (x-hat augmented with ones row)
D_FF = 2 * C         # 640
KT_C = [(0, 128), (128, 256), (256, 321)]       # K tiles over 321 aug channels
CT = [(0, 128), (128, 256), (256, 320)]         # channel tiles of 320
FT = [(0, 128), (128, 256), (256, 384), (384, 512), (512, 640)]  # 640 ff tiles
NPAIR = 4
DA = 41              # v augmented: col 0 = ones (Z row), cols 1..40 = v


# ---------------- host geometry (exact epipolar mask) ----------------

def _host_epipolar_dw(Kin, src_c2w, tgt_c2w):
    """Replicates reference _get_epipolar in f64. Returns dw (b, HW_q, HW_k)."""
    h, w = H, W
    b = Kin.shape[0]
    Wimg = h * 16.0 / 9.0
    K = Kin.astype(np.float64) * np.array([Wimg, float(h), 1.0])[None, :, None]
    K[:, 0, 2] = h / 2.0
    K[:, 1, 2] = h / 2.0
    ii, jj = np.meshgrid(np.arange(h), np.arange(w), indexing='ij')
    coords = np.stack([jj.ravel(), ii.ravel(), np.ones(h * w)], 1).astype(np.float64)
    fx = K[:, 0, 0][:, None]; fy = K[:, 1, 1][:, None]
    cx = K[:, 0, 2][:, None]; cy = K[:, 1, 2][:, None]
    cam = np.stack([(coords[None, :, 0] - cx) / fx,
                    (coords[None, :, 1] - cy) / fy,
                    np.broadcast_to(coords[None, :, 2], (b, h * w))], -1)
    src_r, src_t = src_c2w[:, :3, :3].astype(np.float64), src_c2w[:, :3, 3].astype(np.float64)
    tgt_r_inv = np.linalg.inv(tgt_c2w[:, :3, :3].astype(np.float64))
    tgt_t = -tgt_c2w[:, :3, 3].astype(np.float64)
    p_world = np.einsum('bij,bnj->bni', src_r, cam) + src_t[:, None]
    p_tgt = np.einsum('bij,bnj->bni', tgt_r_inv, p_world) + tgt_t[:, None]
    pi = np.einsum('bij,bnj->bni', K, p_tgt)
    pi = pi / (pi[..., -1:] + 1e-6)
    o_tgt = np.einsum('bij,bj->bi', tgt_r_inv, src_t) + tgt_t
    oi = np.einsum('bij,bj->bi', K, o_tgt)
    oi = (oi / oi[..., -1:])[:, None, :]
    a = pi - oi                       # (b, HWq, 3)
    cmo = coords[None] - oi           # (b, HWk, 3)
    area = np.linalg.norm(np.cross(a[:, :, None, :], cmo[:, None, :, :]), axis=-1)
    vlen = np.linalg.norm(a, axis=-1)
    dist = area / vlen[:, :, None]
    dw = 1.0 - 1.0 / (1.0 + np.exp(-50.0 * (dist - 0.5)))
    row_max = dw.max(-1)
    return np.where((row_max < 0.5)[..., None], 1.0, dw)


def host_E(intrinsic, c2w):
    e1 = _host_epipolar_dw(intrinsic, c2w[1], c2w[0])
    e2 = _host_epipolar_dw(intrinsic, c2w[0], c2w[1]).transpose(0, 2, 1)
    return e1 * e2                    # (B, q, k)


# ---------------- device program ----------------


def build_nc(reps=1, for_sim=False, debug=False):
    nc = bass.Bass(target_bir_lowering=False, debug=False)
    P = {}
    def inp(name, shape, dt):
        P[name] = nc.declare_dram_parameter(name, list(shape), dt, isOutput=False)
        return P[name]
    inp("xq", (C, QH), F32)
    inp("src", (C, HWN), F32)
    inp("E", (8, 128, QH), F16)
    inp("v1", (1, NPAIR * 2 * DA), F16)
    inp("wq", (C_AUG, 512), F16)
    inp("wk", (C_AUG, 512), F16)
    inp("wv", (C_AUG, C), F16)
    inp("wo", (128, NPAIR, C), F16)
    inp("w1", (C_AUG, D_FF), F16)
    inp("w2", (D_FF, C), F16)
    inp("b2p", (128, C), F16)
    inp("wpre", (128, C), F16)
    inp("wpost", (C, 1), F32)
    inp("bpost", (C, 1), F32)
    out = nc.declare_dram_parameter("out", [C, QH], F32, isOutput=True)
    dbg = {}
    if debug:
        for nm, shape, dt in [("d_qT", (128, QH), F16), ("d_kT", (128, HWN), F16),
                              ("d_pke", (128, 8, QH), F16), ("d_pkE", (128, 8, QH), F16),
                              ("d_at", (128, QH), F32), ("d_atn", (128, QH), F16),
                              ("d_v1", (128, 2 * DA), F16), ("d_E", (128, 8, QH), F16),
                              ("d_pke1", (128, 8, QH), F16), ("d_pkE1", (128, 8, QH), F16)]:
            dbg[nm] = nc.declare_dram_parameter(nm, list(shape), dt, isOutput=True)

    with tile.TileContext(nc) as tc:
        for _ in range(reps):
            _emit(nc, tc, P, out, dbg)

    if not for_sim:
        split_multi_waits(nc)
    return nc


def quake_rsqrt(nc, pool, mv, n, tag):
    """rq = 1/sqrt(|mv| + 1e-5) elementwise on [128, n] f32 via Quake + 2 Newton."""
    t = [pool.tile([128, n], F32, name=f"qk{tag}{i}", tag=f"qk{tag}{i}") for i in range(4)]
    a, bb, y, s = t
    nc.vector.tensor_scalar(out=a[:].bitcast(I32), in0=mv[:].bitcast(I32),
                            scalar1=0x7FFFFFFF, scalar2=None, op0=ALU.bitwise_and)
    nc.vector.tensor_scalar(out=bb[:], in0=a[:], scalar1=1e-5, scalar2=None, op0=ALU.add)
    nc.vector.tensor_scalar(out=a[:].bitcast(I32), in0=bb[:].bitcast(I32),
                            scalar1=1, scalar2=None, op0=ALU.logical_shift_right)
    nc.vector.tensor_scalar(out=s[:].bitcast(I32), in0=a[:].bitcast(I32),
                            scalar1=-1, scalar2=None, op0=ALU.bitwise_xor)
    nc.vector.tensor_scalar(out=y[:].bitcast(I32), in0=s[:].bitcast(I32),
                            scalar1=0x5F3759E0, scalar2=None, op0=ALU.add)
    rq = pool.tile([128, n], F32, name=f"qr{tag}", tag=f"qr{tag}")
    d = pool.tile([128, n], F32, name=f"qd{tag}", tag=f"qd{tag}")
    cur = y
    for it in range(2):
        nxt = rq if it == 1 else s
        nc.vector.tensor_tensor(out=a[:], in0=bb[:], in1=cur[:], op=ALU.mult)
        nc.vector.tensor_tensor(out=d[:], in0=a[:], in1=cur[:], op=ALU.mult)
        nc.vector.tensor_scalar(out=a[:], in0=d[:], scalar1=-0.5, scalar2=1.5,
                                op0=ALU.mult, op1=ALU.add)
        nc.vector.tensor_tensor(out=nxt[:], in0=cur[:], in1=a[:], op=ALU.mult)
        cur = nxt
    return rq


def _ln_group(nc, tc, sbT, xp_list, ntiles, tag):
    """Batch LN stats for a list of psum/sbuf [128, C]-f32 tiles. Returns
    (rq, ms) where rstd_t = rq[:, 2t+1], ms_t = ms[:, t] (mean*rstd)."""
    mv = sbT.tile([128, 2 * ntiles], F32, name=f"mv{tag}", tag=f"mv{tag}")
    for t, xp in enumerate(xp_list):
        st6 = sbT.tile([128, 6], F32, name=f"s6{tag}", tag=f"s6{tag}")
        nc.vector.bn_stats(out=st6[:], in_=xp[:])
        nc.vector.bn_aggr(out=mv[:, 2 * t:2 * t + 2], in_=st6[:])
    rq = quake_rsqrt(nc, sbT, mv, 2 * ntiles, tag)
    ms = sbT.tile([128, ntiles], F32, name=f"ms{tag}", tag=f"ms{tag}")
    for t in range(ntiles):
        nc.vector.tensor_tensor(out=ms[:, t:t + 1], in0=mv[:, 2 * t:2 * t + 1],
                                in1=rq[:, 2 * t + 1:2 * t + 2], op=ALU.mult)
    return rq, ms


def _ln_transpose_phase(nc, tc, sbP, sbT, x_sb, npix, ident, ident16, tag, outT):
    """LN per pixel over channels for [C, npix] channel-major sbuf tiles,
    writing transposed-back f16 tiles into outT (aug ones row preset)."""
    ntiles = npix // 128
    xsb = []
    with tc.tile_pool(name=f"pT{tag}", bufs=2, space="PSUM") as pT:
        for pt in range(ntiles):
            xp = pT.tile([128, C], F32, name="xp", tag="xp")
            for ci, (c0, c1) in enumerate(CT):
                cw = c1 - c0
                nc.tensor.transpose(xp[:, c0:c1], x_sb[ci][:, pt * 128:(pt + 1) * 128],
                                    ident[0:cw, 0:cw])
            xs = sbT.tile([128, C], F32, name=f"xs{tag}{pt}", tag=f"xs{tag}{pt}")
            nc.scalar.activation(out=xs[:], in_=xp[:], func=AF.Copy, bias=0.0, scale=1.0)
            xsb.append(xs)
    rq, ms = _ln_group(nc, tc, sbT, xsb, ntiles, tag)
    with tc.tile_pool(name=f"pB{tag}", bufs=2, space="PSUM") as pB:
        for pt in range(ntiles):
            xh = sbT.tile([128, C], F16, name=f"xh{tag}", tag=f"xh{tag}")
            nc.vector.tensor_scalar(out=xh[:], in0=xsb[pt][:],
                                    scalar1=rq[:, 2 * pt + 1:2 * pt + 2],
                                    scalar2=ms[:, pt:pt + 1],
                                    op0=ALU.mult, op1=ALU.subtract)
            for ci, (c0, c1) in enumerate(CT):
                cw = c1 - c0
                tb = pB.tile([128, 128], F16, name="tb", tag="tb")
                nc.tensor.transpose(tb[0:cw, 0:128], xh[:, c0:c1], ident16[:])
                nc.vector.tensor_copy(out=outT[ci][0:cw, pt * 128:(pt + 1) * 128],
                                      in_=tb[0:cw, 0:128])


def _emit(nc, tc, P, out, dbg=None):
    from contextlib import ExitStack
    ctx = ExitStack()
    with ctx:
        consts = ctx.enter_context(tc.tile_pool(name="consts", bufs=1))
        sbW = ctx.enter_context(tc.tile_pool(name="weights", bufs=1))
        sbP = ctx.enter_context(tc.tile_pool(name="persist", bufs=1))
        sbT = ctx.enter_context(tc.tile_pool(name="scratch", bufs=1))
        

        ident = consts.tile([128, 128], F32)
        make_identity(nc, ident[:])
        ident16 = consts.tile([128, 128], F16)
        nc.vector.tensor_copy(out=ident16[:], in_=ident[:])
        onesT = consts.tile([128, DA], F16)
        nc.gpsimd.memset(onesT[:], 1.0)
        ones1 = consts.tile([1, 128], F16)
        nc.gpsimd.memset(ones1[:], 1.0)
        onesRow = consts.tile([128, QH], F16)
        nc.gpsimd.memset(onesRow[:], 0.0)
        nc.gpsimd.memset(onesRow[0:1, :], 1.0)
        IVr0 = consts.tile([128, QH], F32)
        nc.gpsimd.memset(IVr0[:], 0.0)
        IVr1 = consts.tile([128, QH], F32)
        nc.gpsimd.memset(IVr1[:], 0.0)
        onesT32 = consts.tile([128, DA], F32)
        nc.gpsimd.memset(onesT32[:], 1.0)
        # exp act-table preload
        dmi = consts.tile([1, 2], F32)
        nc.gpsimd.memset(dmi[:], 0.0)
        dmo = consts.tile([1, 2], F32)
        nc.scalar.activation(out=dmo[:], in_=dmi[:], func=AF.Exp, bias=0.0, scale=1.0)

        # ---- DMA loads (priority order) ----
        def load(name, shape, dt, pool=sbW):
            t = pool.tile(list(shape), dt, name=name, tag=name)
            nc.sync.dma_start(out=t[:], in_=P[name][:])
            return t

        sbIn = ctx.enter_context(tc.tile_pool(name="inputs", bufs=1))
        xq_sb = []
        for ci, (c0, c1) in enumerate(CT):
            t = sbIn.tile([c1 - c0, QH], F32, name=f"xq{ci}", tag=f"xq{ci}")
            nc.sync.dma_start(out=t[:, 0:256], in_=P["xq"][c0:c1, 0:256])
            nc.sync.dma_start(out=t[:, 256:512], in_=P["xq"][c0:c1, 256:512])
            xq_sb.append(t)
        wq_sb = [load_slice(nc, sbW, P["wq"], kt, 512, F16) for kt in KT_C]
        src_sb = []
        for ci, (c0, c1) in enumerate(CT):
            t = sbIn.tile([c1 - c0, HWN], F32, name=f"src{ci}", tag=f"src{ci}")
            for h4 in range(4):
                nc.sync.dma_start(out=t[:, h4 * 256:(h4 + 1) * 256],
                                  in_=P["src"][c0:c1, h4 * 256:(h4 + 1) * 256])
            src_sb.append(t)
        wk_sb = [load_slice(nc, sbW, P["wk"], kt, 512, F16) for kt in KT_C]
        wv_sb = [load_slice(nc, sbW, P["wv"], kt, C, F16) for kt in KT_C]
        E8 = sbP.tile([128, 8, QH], F16, name="E8", tag="E8")
        for kt in range(8):
            nc.sync.dma_start(out=E8[:, kt, :], in_=P["E"][kt])
        V1sb = sbW.tile([128, NPAIR * 2 * DA], F16, name="V1", tag="V1")
        nc.gpsimd.memset(V1sb[:], 0.0)
        nc.sync.dma_start(out=V1sb[0:1, :], in_=P["v1"][:])
        wo_sb = []
        for g in range(NPAIR):
            t = sbW.tile([128, C], F16, name=f"wo{g}", tag=f"wo{g}")
            nc.sync.dma_start(out=t[:], in_=P["wo"][g])
            wo_sb.append(t)
        w1_sb = [load_slice(nc, sbW, P["w1"], kt, D_FF, F16) for kt in KT_C]
        w2_sb = []
        for ft in FT:
            t = sbW.tile([128, C], F16, name=f"w2{ft[0]}", tag=f"w2{ft[0]}")
            nc.sync.dma_start(out=t[:], in_=P["w2"][ft[0]:ft[1], :])
            w2_sb.append(t)
        b2p = load("b2p", (128, C), F16)
        wpre = load("wpre", (128, C), F16)
        wpost_sb = [load_slice(nc, sbW, P["wpost"], ct, 1, F32) for ct in CT]
        bpost_sb = [load_slice(nc, sbW, P["bpost"], ct, 1, F32) for ct in CT]

        # ---- Phase LN+transpose ----
        xqT = [sbP.tile([k1 - k0, QH], F16, name=f"xqT{i}", tag=f"xqT{i}")
               for i, (k0, k1) in enumerate(KT_C)]
        srcT = [sbP.tile([k1 - k0, HWN], F16, name=f"srT{i}", tag=f"srT{i}")
                for i, (k0, k1) in enumerate(KT_C)]
        nc.gpsimd.memset(xqT[2][64:65, :], 1.0)
        nc.gpsimd.memset(srcT[2][64:65, :], 1.0)
        _ln_transpose_phase(nc, tc, sbP, sbT, xq_sb, QH, ident, ident16, "xq", xqT)
        _ln_transpose_phase(nc, tc, sbP, sbT, src_sb, HWN, ident, ident16, "sr", srcT)

        # ---- Phase projections (pair-packed q/k; per-ktile v) ----
        qT2 = [sbP.tile([128, QH], F16, name=f"qT{g}", tag=f"qT{g}") for g in range(NPAIR)]
        kT2 = [sbP.tile([128, HWN], F16, name=f"kT{g}", tag=f"kT{g}") for g in range(NPAIR)]
        v_sb = [sbP.tile([128, HEADS, DA], F16, name=f"v{kt}", tag=f"v{kt}")
                for kt in range(8)]
        with (
            tc.tile_pool(name="pq", bufs=2, space="PSUM") as pq,
            tc.tile_pool(name="pv", bufs=2, space="PSUM") as pv,
        ):
            for g in range(NPAIR):
                qp = pq.tile([128, QH], F32, name="qp", tag="qp")
                for ki in range(3):
                    nc.tensor.matmul(qp[:], wq_sb[ki][:, g * 128:(g + 1) * 128], xqT[ki][:],
                                     start=(ki == 0), stop=(ki == 2))
                nc.scalar.activation(out=qT2[g][:], in_=qp[:], func=AF.Copy, bias=0.0, scale=1.0)
                for half in range(2):
                    kp = pq.tile([128, QH], F32, name="qp", tag="qp")
                    for ki in range(3):
                        nc.tensor.matmul(kp[:], wk_sb[ki][:, g * 128:(g + 1) * 128],
                                         srcT[ki][:, half * 512:(half + 1) * 512],
                                         start=(ki == 0), stop=(ki == 2))
                    nc.scalar.activation(out=kT2[g][:, half * 512:(half + 1) * 512],
                                         in_=kp[:], func=AF.Copy, bias=0.0, scale=1.0)
            for kt in range(8):
                vp = pv.tile([128, C], F32, name="vp", tag="vp")
                for ki in range(3):
                    nc.tensor.matmul(vp[:], srcT[ki][:, kt * 128:(kt + 1) * 128], wv_sb[ki][:],
                                     start=(ki == 0), stop=(ki == 2))
                nc.gpsimd.memset(v_sb[kt][:, :, 0:1], 1.0)
                nc.scalar.activation(out=v_sb[kt][:, :, 1:DA],
                                     in_=vp[:].rearrange("p (h d) -> p h d", h=HEADS),
                                     func=AF.Copy, bias=0.0, scale=1.0)

        # ---- EV = sum_kt v^T E per head (3 stacked-head groups) ----
        EVsb = []
        with tc.tile_pool(name="pEV", bufs=1, space="PSUM") as pEV:
            for g3 in range(3):
                nh = 3 if g3 < 2 else 2
                rows = DA * nh
                ev = pEV.tile([128, QH], F32, name=f"ev{g3}", tag=f"ev{g3}")
                for kt in range(8):
                    nc.tensor.matmul(
                        ev[0:rows, :],
                        v_sb[kt][:, 3 * g3:3 * g3 + nh, :].rearrange("p h d -> p (h d)"),
                        E8[:, kt, :], start=(kt == 0), stop=(kt == 7))
                evs = sbP.tile([128, QH], F16, name=f"evs{g3}", tag=f"evs{g3}")
                nc.gpsimd.memset(evs[:], 0.0)
                nc.vector.tensor_scalar(out=evs[0:rows, :], in0=ev[0:rows, :],
                                        scalar1=-1.0, scalar2=None, op0=ALU.mult)
                EVsb.append(evs)

        # ---- Phase attention ----
        atn2 = [sbP.tile([128, QH], F16, name=f"atn{g}", tag=f"atn{g}") for g in range(NPAIR)]
        if dbg:
            for g in range(NPAIR):
                nc.gpsimd.memset(atn2[g][:], 0.0)
        with (
            tc.tile_pool(name="psSt", bufs=2, space="PSUM") as psSt,
            tc.tile_pool(name="psAt", bufs=2, space="PSUM") as psAt,
            tc.tile_pool(name="psZb", bufs=2, space="PSUM") as psZb,
            tc.tile_pool(name="pke", bufs=2) as pkeP,
            tc.tile_pool(name="blend", bufs=2) as blP,
        ):
            for g in range(NPAIR):
                h0, h1 = 2 * g, 2 * g + 1
                pke = {h0: pkeP.tile([128, 8, QH], F16, name="pke", tag="pke"),
                       h1: pkeP.tile([128, 8, QH], F16, name="pke", tag="pke")}
                # row-tiled QK^T, 2 kt per psum tile, exp from psum
                for c in range(4):
                    kts = 2 * c
                    sA = psSt.tile([128, 2, QH], F32, name="st", tag="st")
                    sB = psSt.tile([128, 2, QH], F32, name="st", tag="st")
                    for j in range(2):
                        kt = kts + j
                        nc.tensor.matmul(sA[:, j, :], kT2[g][0:40, kt * 128:(kt + 1) * 128],
                                         qT2[g][0:40, :], start=True, stop=True)
                        nc.tensor.matmul(sB[:, j, :], kT2[g][64:104, kt * 128:(kt + 1) * 128],
                                         qT2[g][64:104, :], start=True, stop=True)
                    nc.scalar.activation(out=pke[h0][:, kts:kts + 2, :], in_=sA[:],
                                         func=AF.Exp, bias=0.0, scale=1.0)
                    nc.scalar.activation(out=pke[h1][:, kts:kts + 2, :], in_=sB[:],
                                         func=AF.Exp, bias=0.0, scale=1.0)
                # blend: pkE = pke * E  (the "-1*E" is folded in via the EV matmul)
                pkE = {}
                for h in (h0, h1):
                    pe = blP.tile([128, 8, QH], F16, name="pe", tag="bl")
                    nc.vector.tensor_tensor(out=pe[:], in0=pke[h][:], in1=E8[:], op=ALU.mult)
                    pkE[h] = pe
                # col-tiled PV + Z row; V1 init carries the "+1".
                # Only the FIRST matmul into the bank has start=True: a second
                # start would clear the whole bank's has_written bits and turn
                # the first group's later accumulations into overwrites.
                at2a = psAt.tile([128, QH], F32, name="at2a", tag="at2")
                at2b = psAt.tile([128, QH], F32, name="at2b", tag="at2")
                g0, g1x = g * 2 * DA, g * 2 * DA + DA
                nc.tensor.matmul(at2a[0:DA, :], V1sb[:, g0:g0 + DA], onesRow[:],
                                 start=True, stop=False, skip_group_check=True)
                nc.tensor.matmul(at2b[64:64 + DA, :], V1sb[:, g1x:g1x + DA], onesRow[:],
                                 start=True, stop=False, skip_group_check=True)
                j0, q0g = h0 % 3, h0 // 3
                j1, q1g = h1 % 3, h1 // 3
                nc.tensor.matmul(at2a[0:DA, :], ident16[:, DA * j0:DA * j0 + DA],
                                 EVsb[q0g][:], start=False, stop=False, skip_group_check=True)
                nc.tensor.matmul(at2b[64:64 + DA, :], ident16[:, DA * j1:DA * j1 + DA],
                                 EVsb[q1g][:], start=False, stop=False, skip_group_check=True)
                for kt in range(8):
                    last = kt == 7
                    nc.tensor.matmul(at2a[0:DA, :], v_sb[kt][:, h0, :], pkE[h0][:, kt, :],
                                     start=False, stop=last, skip_group_check=True)
                    nc.tensor.matmul(at2b[64:64 + DA, :], v_sb[kt][:, h1, :], pkE[h1][:, kt, :],
                                     start=False, stop=last, skip_group_check=True)
                nc.vector.reciprocal(out=IVr0[0:1, :], in_=at2a[0:1, :])
                nc.vector.reciprocal(out=IVr1[64:65, :], in_=at2b[64:65, :])
                zb2a = psZb.tile([128, QH], F32, name="zb2a", tag="zb2")
                zb2b = psZb.tile([128, QH], F32, name="zb2b", tag="zb2")
                nc.tensor.matmul(zb2a[0:DA, :], onesT32[:, 0:DA], IVr0[:],
                                 start=True, stop=True, skip_group_check=True)
                nc.tensor.matmul(zb2b[64:64 + DA, :], onesT32[:, 0:DA], IVr1[:],
                                 start=True, stop=True, skip_group_check=True)
                zbs = sbT.tile([128, QH], F16, name="zbs", tag="zbs")
                nc.scalar.activation(out=zbs[0:DA, :], in_=zb2a[0:DA, :],
                                     func=AF.Copy, bias=0.0, scale=1.0)
                nc.scalar.activation(out=zbs[64:64 + DA, :], in_=zb2b[64:64 + DA, :],
                                     func=AF.Copy, bias=0.0, scale=1.0)
                atE = sbT.tile([128, QH], F16, name="atE", tag="atE")
                nc.scalar.activation(out=atE[0:DA, :], in_=at2a[0:DA, :],
                                     func=AF.Copy, bias=0.0, scale=1.0)
                nc.scalar.activation(out=atE[64:64 + DA, :], in_=at2b[64:64 + DA, :],
                                     func=AF.Copy, bias=0.0, scale=1.0)
                nc.vector.tensor_tensor(out=atn2[g][0:DA, :], in0=atE[0:DA, :],
                                        in1=zbs[0:DA, :], op=ALU.mult)
                nc.vector.tensor_tensor(out=atn2[g][64:64 + DA, :], in0=atE[64:64 + DA, :],
                                        in1=zbs[64:64 + DA, :], op=ALU.mult)
                if dbg and g == 0:
                    for nm, src in [("d_qT", qT2[0][:]), ("d_kT", kT2[0][:]),
                                    ("d_pke", pke[0][:]), ("d_pkE", pkE[0][:]),
                                    ("d_E", E8[:]), ("d_atn", atn2[0][:]),
                                    ("d_v1", V1sb[:, 0:2 * DA]),
                                    ("d_pke1", pke[1][:]), ("d_pkE1", pkE[1][:])]:
                        nc.gpsimd.dma_start(out=dbg[nm][:], in_=src)
                    dA = sbT.tile([128, QH], F32, name="dA", tag="dA")
                    nc.gpsimd.memset(dA[:], 0.0)
                    nc.vector.tensor_copy(out=dA[0:DA, :], in_=at2a[0:DA, :])
                    nc.vector.tensor_copy(out=dA[64:64 + DA, :], in_=at2b[64:64 + DA, :])
                    nc.gpsimd.dma_start(out=dbg["d_at"][:], in_=dA[:])

        # ---- Phase Wo (row-tiled over head pairs) + LN_pre ----
        resid1 = [sbP.tile([128, C], F16, name=f"res{pt}", tag=f"res{pt}") for pt in range(4)]
        zT = [sbP.tile([k1 - k0, QH], F16, name=f"zT{i}", tag=f"zT{i}")
              for i, (k0, k1) in enumerate(KT_C)]
        nc.gpsimd.memset(zT[2][64:65, :], 1.0)
        ysb = []
        with tc.tile_pool(name="psY", bufs=4, space="PSUM") as psY:
            for pt in range(4):
                yA = psY.tile([128, C], F32, name="yA", tag="y")
                yB = psY.tile([128, C], F32, name="yB", tag="y")
                for g in range(NPAIR):
                    nc.tensor.matmul(yA[:], atn2[g][0:DA, pt * 128:(pt + 1) * 128],
                                     wo_sb[g][0:DA, :], start=(g == 0), stop=(g == 3),
                                     skip_group_check=True)
                    nc.tensor.matmul(yB[:], atn2[g][64:64 + DA, pt * 128:(pt + 1) * 128],
                                     wo_sb[g][64:64 + DA, :], start=(g == 0), stop=(g == 3),
                                     skip_group_check=True)
                yBs = sbT.tile([128, C], F32, name="yBs", tag="yBs")
                nc.vector.tensor_copy(out=yBs[:], in_=yB[:])
                ys = sbT.tile([128, C], F32, name=f"ys{pt}", tag=f"ys{pt}")
                nc.vector.tensor_tensor(out=ys[:], in0=yA[:], in1=yBs[:], op=ALU.add)
                ysb.append(ys)
        rqP, msP = _ln_group(nc, tc, sbT, ysb, 4, "pre")
        with tc.tile_pool(name="pZt", bufs=2, space="PSUM") as pZt:
            for pt in range(4):
                zh = sbT.tile([128, C], F16, name=f"zh{pt}", tag=f"zh{pt}")
                nc.vector.tensor_scalar(out=zh[:], in0=ysb[pt][:],
                                        scalar1=rqP[:, 2 * pt + 1:2 * pt + 2],
                                        scalar2=msP[:, pt:pt + 1],
                                        op0=ALU.mult, op1=ALU.subtract)
                nc.vector.tensor_tensor(out=resid1[pt][:], in0=zh[:], in1=wpre[:], op=ALU.mult)
                for ci, (c0, c1) in enumerate(CT):
                    cw = c1 - c0
                    tp = pZt.tile([128, 128], F16, name="tp", tag="tp")
                    nc.tensor.transpose(tp[0:cw, 0:128], zh[:, c0:c1], ident16[:])
                    nc.vector.tensor_copy(out=zT[ci][0:cw, pt * 128:(pt + 1) * 128],
                                          in_=tp[0:cw, 0:128])

        # ---- Phase MLP + LN_post + out ----
        g1 = [sbP.tile([128, QH], F16, name=f"g1{mt}", tag=f"g1{mt}") for mt in range(5)]
        rsb = []
        with (
            tc.tile_pool(name="psM", bufs=2, space="PSUM") as psM,
            tc.tile_pool(name="psV", bufs=1, space="PSUM") as psV,
        ):
            for mt, (f0, f1) in enumerate(FT):
                h1p = psM.tile([128, QH], F32, name="h1", tag="h1")
                for ki in range(3):
                    nc.tensor.matmul(h1p[:], w1_sb[ki][:, f0:f1], zT[ki][:],
                                     start=(ki == 0), stop=(ki == 2))
                nc.scalar.activation(out=g1[mt][:], in_=h1p[:], func=AF.Gelu, bias=0.0, scale=1.0)
            for pt in range(4):
                mp = psM.tile([128, C], F32, name="mp", tag="mp")
                for mt in range(5):
                    nc.tensor.matmul(mp[:], g1[mt][:, pt * 128:(pt + 1) * 128], w2_sb[mt][:],
                                     start=(mt == 0), stop=(mt == 4))
                nc.tensor.matmul(mp[:], ident16[:], resid1[pt][:],
                                 start=False, stop=False, skip_group_check=True)
                nc.tensor.matmul(mp[:], ident16[:], b2p[:],
                                 start=False, stop=True, skip_group_check=True)
                rs = sbT.tile([128, C], F32, name=f"rs{pt}", tag=f"rs{pt}")
                nc.scalar.activation(out=rs[:], in_=mp[:], func=AF.Copy, bias=0.0, scale=1.0)
                rsb.append(rs)
            rqF, msF = _ln_group(nc, tc, sbT, rsb, 4, "post")
            vt_ps = [psV.tile([128, QH], F16, name=f"vt{ci}", tag=f"vt{ci}") for ci in range(3)]
            for pt in range(4):
                vh = sbT.tile([128, C], F16, name=f"vh{pt}", tag=f"vh{pt}")
                nc.vector.tensor_scalar(out=vh[:], in0=rsb[pt][:],
                                        scalar1=rqF[:, 2 * pt + 1:2 * pt + 2],
                                        scalar2=msF[:, pt:pt + 1],
                                        op0=ALU.mult, op1=ALU.subtract)
                for ci, (c0, c1) in enumerate(CT):
                    cw = c1 - c0
                    nc.tensor.transpose(vt_ps[ci][0:cw, pt * 128:(pt + 1) * 128],
                                        vh[:, c0:c1], ident16[:])
            for ci, (c0, c1) in enumerate(CT):
                cw = c1 - c0
                o_sb = sbT.tile([128, QH], F32, name="osb", tag="osb")
                nc.vector.tensor_scalar(out=o_sb[0:cw, :], in0=vt_ps[ci][0:cw, :],
                                        scalar1=wpost_sb[ci][0:cw, :],
                                        scalar2=bpost_sb[ci][0:cw, :],
                                        op0=ALU.mult, op1=ALU.add)
                nc.sync.dma_start(out=out[c0:c1, 0:256], in_=o_sb[0:cw, 0:256])
                nc.sync.dma_start(out=out[c0:c1, 256:512], in_=o_sb[0:cw, 256:512])


def load_slice(nc, pool, param, kt, ncols, dt):
    k0, k1 = kt
    t = pool.tile([k1 - k0, ncols], dt, name=f"ld{param.name}{k0}", tag=f"ld{param.name}{k0}")
    nc.sync.dma_start(out=t[:], in_=param[k0:k1, :])
    return t


# ================= host side =================

def host_prep(x, src_encode, intrinsic, c2w, ln_q_w, ln_q_b, Wq, bq, ln_k_w, ln_k_b,
              Wk, bk, ln_v_w, ln_v_b, Wv, bv, Wo, bo, ln_pre_w, ln_pre_b, W1, b1,
              W2, b2, ln_post_w, ln_post_b):
    """Returns list of 8 in_maps."""
    f64 = np.float64
    f16 = ml_dtypes.float16 if hasattr(ml_dtypes, 'float16') else np.float16

    E_full = host_E(intrinsic, c2w)                    # (B, q, k) f64

    Wq_f = (np.diag(ln_q_w.astype(f64)) @ Wq.astype(f64)) * SCALE
    bq_f = (ln_q_b.astype(f64) @ Wq.astype(f64) + bq) * SCALE
    Wk_f = np.diag(ln_k_w.astype(f64)) @ Wk.astype(f64)
    bk_f = ln_k_b.astype(f64) @ Wk.astype(f64) + bk
    Wv_f = np.diag(ln_v_w.astype(f64)) @ Wv.astype(f64)
    bv_f = ln_v_b.astype(f64) @ Wv.astype(f64) + bv
    W1_f = np.diag(ln_pre_w.astype(f64)) @ W1.astype(f64)
    b1_f = ln_pre_b.astype(f64) @ W1.astype(f64) + b1
    b2p = b2.astype(f64) + ln_pre_b.astype(f64)

    def aug(Wf, bf):
        return np.concatenate([Wf, bf[None, :]], 0)

    def pack_pairs(w_aug):
        # (321, 320) -> (321, 512): per pair g cols [40 h0 | 24 zero | 40 h1 | 24 zero]
        o = np.zeros((C_AUG, 512), f64)
        for g in range(NPAIR):
            o[:, g * 128:g * 128 + 40] = w_aug[:, (2 * g) * 40:(2 * g + 1) * 40]
            o[:, g * 128 + 64:g * 128 + 104] = w_aug[:, (2 * g + 1) * 40:(2 * g + 2) * 40]
        return o

    wq_np = pack_pairs(aug(Wq_f, bq_f)).astype(np.float16)
    wk_np = pack_pairs(aug(Wk_f, bk_f)).astype(np.float16)
    wv_np = aug(Wv_f, bv_f).astype(np.float16)
    wo_np = np.zeros((NPAIR, 128, C), f64)
    for g in range(NPAIR):
        wo_np[g, 1:DA, :] = Wo.astype(f64)[(2 * g) * 40:(2 * g + 1) * 40, :]
        wo_np[g, 65:64 + DA, :] = Wo.astype(f64)[(2 * g + 1) * 40:(2 * g + 2) * 40, :]
    wo_np[0, 0, :] = bo
    wo_np = np.ascontiguousarray(wo_np.astype(np.float16).transpose(1, 0, 2))
    w1_np = aug(W1_f, b1_f).astype(np.float16)
    w2_np = W2.astype(np.float16)
    b2p_np = np.broadcast_to(b2p[None, :], (128, C)).astype(np.float16).copy()
    wpre_np = np.broadcast_to(ln_pre_w[None, :], (128, C)).astype(np.float16).copy()
    wpost_np = ln_post_w[:, None].astype(np.float32).copy()
    bpost_np = ln_post_b[:, None].astype(np.float32).copy()

    # V1 = sum_k v_aug per head (exact, f64)
    src_f = src_encode.reshape(B, C, HWN).transpose(0, 2, 1).astype(f64)  # (B, k, C)
    m = src_f.mean(-1, keepdims=True)
    v = ((src_f - m) ** 2).mean(-1, keepdims=True)
    ln_src = (src_f - m) / np.sqrt(v + 1e-5)
    v_all = ln_src @ Wv_f + bv_f                       # (B, k, 320)
    v_sum = v_all.sum(1)                               # (B, 320)
    v1_np = np.zeros((B, NPAIR, 2 * DA), f64)
    for g in range(NPAIR):
        v1_np[:, g, 0] = float(HWN)
        v1_np[:, g, 1:DA] = v_sum[:, (2 * g) * 40:(2 * g + 1) * 40]
        v1_np[:, g, DA] = float(HWN)
        v1_np[:, g, DA + 1:2 * DA] = v_sum[:, (2 * g + 1) * 40:(2 * g + 2) * 40]
    v1_np = v1_np.astype(np.float16)

    in_maps = []
    for core in range(8):
        b = core // 2
        half = core % 2
        q0, q1 = half * QH, (half + 1) * QH
        E_core = np.ascontiguousarray(
            E_full[b][q0:q1, :].T.reshape(8, 128, QH).astype(np.float16))
        im = {
            "xq": np.ascontiguousarray(x[b].reshape(C, HWN)[:, q0:q1].astype(np.float32)),
            "src": np.ascontiguousarray(src_encode[b].reshape(C, HWN).astype(np.float32)),
            "E": E_core,
            "v1": v1_np[b].reshape(1, NPAIR * 2 * DA),
            "wq": wq_np, "wk": wk_np, "wv": wv_np, "wo": wo_np,
            "w1": w1_np, "w2": w2_np, "b2p": b2p_np,
            "wpre": wpre_np, "wpost": wpost_np, "bpost": bpost_np,
        }
        in_maps.append(im)
    return in_maps


def assemble(results):
    out = np.zeros((B, C, HWN), np.float32)
    for core in range(8):
        b, half = core // 2, core % 2
        out[b][:, half * QH:(half + 1) * QH] = results[core]["out"]
    return out.reshape(B, C, H, W)


_CACHE = {}

def kernel(**inputs):
    from concourse.bass_utils import run_bass_kernel_spmd
    if "nc" not in _CACHE:
        _CACHE["nc"] = build_nc()
    nc = _CACHE["nc"]
    in_maps = host_prep(**inputs)
    res = run_bass_kernel_spmd(nc, in_maps, core_ids=list(range(8)))
    return assemble(res.results)
